# revision 11
# baseline (speedup 1.0000x reference)
"""CFConv (GNN message passing) on 8 Trainium2 cores.

    y = segment_sum(x[idx_j] * Wij, idx_i)   with idx_i sorted

Device strategy (uniform SPMD program, per-core data):
  - Edges sharded contiguously across 8 cores (idx_i sorted => contiguous
    atom ranges; boundary overlaps fixed host-side).
  - Per core, edges are packed into 512-edge "halves" (atom span <= 64,
    verified; pad slots inserted where needed). Each half's 512 slots are
    split into exactly 128 slots per x-"window" (4 overlapping windows of
    32768 rows, stride 25000, wraparound replica) so the gather can use the
    custom int16 dma_gather instruction (4 SWDGE queues).
  - x windows stored in DRAM as bf16 rows PADDED to 256B (row stride 128
    bf16, 64 useful). The gather uses elem_size=64 bf16 (128B descriptors)
    with elem_step=128 (256B stride, the ucode's granularity): half the DMA
    bus time of f32 rows. Gather calls are one per (chunk, window) with
    nh*128 indices (up to 4096) to amortize the ~1us fixed GPSIMD
    descriptor-generation overhead per call.
  - x_j * Wij elementwise on VectorE in bf16 (Wij host-relaid into slot
    order, bf16).
  - Segment-sum via one-hot matmul in bf16: rr = idx_i - half_base in
    [0,64); VectorE builds one-hot (rr == iota) in bf16; TensorE
    accumulates K=128 bf16 matmuls into per-half [64,64] f32 PSUM frames
    (2 frames per 128-row psum block).
  - Device emits dense per-group f32 partials; host adds each [64,F] frame
    into y at its half's base atom (~2 overlapping partials per atom).
"""

import sys

import numpy as np

if "/opt/trn_rl_repo" not in sys.path:
    sys.path.insert(0, "/opt/trn_rl_repo")

import ml_dtypes

BF16 = ml_dtypes.bfloat16

CFG = dict(
    N_ATOMS=100000,
    F=64,
    E=1250000,
    NCORES=8,
    HALF=512,          # edges per half-group
    OHW=64,            # one-hot width (max atom span per half; data max ~50)
    NW=4,              # x windows
    WSTRIDE=25000,     # window stride (NW*WSTRIDE == N_ATOMS)
    WREACH=32768,      # window size (int16 index limit)
    XPAD=128,          # bf16 row stride in padded x windows (256B)
    NH=308,            # halves per core
    # sums to NH; nh*128 idxs per gather call. Mid chunks large to amortize
    # the ~0.5us fixed GPSIMD cost per gather call; last chunks small so the
    # post-gather compute tail after the final gather is short.
    CHUNK_HALVES=[32] * 9 + [12, 8],
)

_CACHE = {}
last_results = None


def _derived(cfg):
    d = dict(cfg)
    d["CAP"] = cfg["NH"] * cfg["HALF"]
    d["NCOLS"] = d["CAP"] // 128
    d["NGROUPS"] = cfg["NH"] // 2
    d["FLEX"] = cfg["WREACH"] - cfg["WSTRIDE"]
    d["IDXCOLS"] = sum(4 * (nh * 128) // 16 for nh in cfg["CHUNK_HALVES"])
    assert sum(cfg["CHUNK_HALVES"]) == cfg["NH"]
    assert cfg["NW"] * cfg["WSTRIDE"] == cfg["N_ATOMS"]
    return d


def _dma_gather_strided(eng, out_ap, in_ap, idxs_ap, num_idxs, elem_size,
                        elem_step, queue_num):
    """nc.gpsimd.dma_gather (non-transpose, DRAM source) without the
    elem_size_bytes % 256 restriction. The ucode's non-transpose path only
    needs the row STRIDE 256B-granular (stride_bytes_256 encoding); the
    payload per descriptor can be any size (here 128B = one bf16 row)."""
    import concourse.mybir as mybir
    from concourse._compat import exact_div

    eng._assert_queue_num(queue_num)
    assert idxs_ap.dtype == mybir.dt.int16
    assert in_ap.dtype == out_ap.dtype
    assert in_ap.ap[0][0] == elem_step
    assert in_ap.ap[-1][1] == out_ap.ap[-1][1] == elem_size
    assert out_ap.ap[0][1] * out_ap.ap[1][1] == ((num_idxs + 127) // 128) * 128
    stride_bytes = elem_step * mybir.dt.size(in_ap.dtype)
    stride_bytes_256 = exact_div(stride_bytes, 256)
    assert stride_bytes_256 < 256
    _in_ap = eng.lower_ap_dma(in_ap, for_custom_bir_dma=True)
    _idxs_ap = eng.lower_ap(idxs_ap)
    _out_ap = eng.lower_ap(out_ap)
    return eng.add_instruction(
        mybir.InstDMAGatherAnt(
            name=eng.bass.get_next_instruction_name(),
            ins=[
                *_in_ap,
                _idxs_ap,
                eng.lower_val_access(eng.to_reg(num_idxs)),
            ],
            outs=[_out_ap],
            transpose=False,
            num_idxs=num_idxs,
            elem_size=elem_size,
            stride_bytes_256=stride_bytes_256,
            gen_mode=0,
            single_packet=False,
            queue_num=queue_num,
        )
    )


def _build_program(cfg):
    import concourse.bacc as bacc
    import concourse.tile as tile
    import concourse.mybir as mybir
    from concourse.library_config import mlp

    d = _derived(cfg)
    F, NW, XPAD = cfg["F"], cfg["NW"], cfg["XPAD"]
    NCOLS, IDXCOLS, NGROUPS = d["NCOLS"], d["IDXCOLS"], d["NGROUPS"]
    WREACH, OHW = cfg["WREACH"], cfg["OHW"]

    nc = bacc.Bacc(
        "TRN2",
        target_bir_lowering=False,
        num_swdge_queues=4,
        dynamic_dma_scratch_size=32768,
    )
    f32 = mybir.dt.float32
    bf16 = mybir.dt.bfloat16
    wij_d = nc.dram_tensor("wij", [128, NCOLS * F], bf16, kind="ExternalInput")
    win_d = [
        nc.dram_tensor(f"w{k}", [WREACH, XPAD], bf16, kind="ExternalInput")
        for k in range(NW)
    ]
    idx_d = nc.dram_tensor("idx16", [128, IDXCOLS], mybir.dt.int16, kind="ExternalInput")
    rr_d = nc.dram_tensor("rr", [128, NCOLS], bf16, kind="ExternalInput")
    iota_d = nc.dram_tensor("iota", [128, OHW], bf16, kind="ExternalInput")
    out_d = nc.dram_tensor("out", [128, NGROUPS * F], f32, kind="ExternalOutput")

    with tile.TileContext(nc) as tc:
        with (
            tc.tile_pool(name="const", bufs=1) as cpool,
            tc.tile_pool(name="idx", bufs=3) as ipool,
            tc.tile_pool(name="wij", bufs=3) as wpool,
            tc.tile_pool(name="gx", bufs=3) as gpool,
            tc.tile_pool(name="oh", bufs=3) as spool,
            tc.tile_pool(name="stage", bufs=3) as opool,
            tc.tile_pool(name="psum", bufs=3, space="PSUM") as ppool,
        ):
            with tc.tile_critical():
                nc.gpsimd.load_library(mlp)

            iota_t = cpool.tile([128, OHW], bf16)
            nc.sync.dma_start(out=iota_t[:], in_=iota_d[:])
            rr_t = cpool.tile([128, NCOLS], bf16)
            nc.sync.dma_start(out=rr_t[:], in_=rr_d[:])

            iota_b = iota_t[:].rearrange("p (o f) -> p o f", o=1)

            col0 = 0   # global column base of chunk
            ix0 = 0    # global idx16 column base
            for nh in cfg["CHUNK_HALVES"]:
                ncols = 4 * nh              # columns in this chunk
                num = nh * 128              # indices per gather call
                icols = num // 16           # idx16 cols per window call
                idx_t = ipool.tile([128, 4 * icols], mybir.dt.int16, tag="idx")
                nc.sync.dma_start(out=idx_t[:], in_=idx_d[:, ix0 : ix0 + 4 * icols])
                wij_sb = wpool.tile([128, ncols * F], bf16, tag="wij")
                nc.sync.dma_start(
                    out=wij_sb[:],
                    in_=wij_d[:, col0 * F : (col0 + ncols) * F],
                )
                gx = gpool.tile([128, ncols * F], bf16, tag="gx")
                for k in range(NW):
                    _dma_gather_strided(
                        nc.gpsimd,
                        out_ap=gx[:, (k * nh) * F : (k + 1) * nh * F].rearrange(
                            "p (c f) -> p c f", f=F
                        ),
                        in_ap=win_d[k][:, 0:F],
                        idxs_ap=idx_t[:, k * icols : (k + 1) * icols],
                        num_idxs=num,
                        elem_size=F,
                        elem_step=XPAD,
                        queue_num=k,
                    )
                # one-hot first: it does not depend on the gather, so the
                # Vector engine can run ahead of the gather stream
                s_t = spool.tile([128, ncols * OHW], bf16, tag="oh")
                nc.vector.tensor_tensor(
                    out=s_t[:],
                    in0=iota_b.to_broadcast([128, ncols, OHW]),
                    in1=rr_t[:, col0 : col0 + ncols].to_broadcast([128, ncols, OHW]),
                    op=mybir.AluOpType.is_equal,
                )
                nc.vector.tensor_tensor(
                    out=gx[:],
                    in0=gx[:],
                    in1=wij_sb[:],
                    op=mybir.AluOpType.mult,
                )
                pt = ppool.tile([128, (nh // 2) * F], f32, tag="ps")
                for hl in range(nh):
                    for k in range(NW):
                        c = k * nh + hl
                        nc.tensor.matmul(
                            out=pt[
                                (hl % 2) * OHW : (hl % 2 + 1) * OHW,
                                (hl // 2) * F : (hl // 2 + 1) * F,
                            ],
                            lhsT=s_t[:, c * OHW : (c + 1) * OHW],
                            rhs=gx[:, c * F : (c + 1) * F],
                            start=(k == 0),
                            stop=(k == NW - 1),
                        )
                stage = opool.tile([128, (nh // 2) * F], f32, tag="st")
                nc.scalar.copy(out=stage[:], in_=pt[:])
                g0 = col0 // 8  # global group base (col0 = sum 4*nh, groups nh/2)
                nc.sync.dma_start(
                    out=out_d[:, g0 * F : (g0 + nh // 2) * F],
                    in_=stage[:],
                )
                col0 += ncols
                ix0 += 4 * icols

    nc.compile()
    return nc


def _solve_half(ai, aj, cfg, take):
    """Pick the largest prefix (<= take) of this half's candidate edges that
    satisfies span<OHW and the per-window capacity-128 balance; returns
    (n_taken, win_assign[n]) or reduces take."""
    WSTRIDE, FLEX, NW, OHW = cfg["WSTRIDE"], cfg["WREACH"] - cfg["WSTRIDE"], cfg["NW"], cfg["OHW"]
    while take > 0:
        a = ai[:take]
        if a[-1] - a[0] >= OHW:
            # cut to span
            take = int(np.searchsorted(a, a[0] + OHW, side="left"))
            continue
        j = aj[:take]
        k = j // WSTRIDE
        fl = (j % WSTRIDE) < FLEX
        e = np.zeros(NW, np.int64)
        f = np.zeros(NW, np.int64)
        for kk in range(NW):
            e[kk] = int(((k == kk) & ~fl).sum())
            f[kk] = int(((k == kk) & fl).sum())
        if e.max() > 128:
            take -= 1
            continue
        sol = None
        for a0 in range(int(f[0]) + 1):
            a1 = max(0, e[0] + a0 + f[1] - 128)
            if a1 > f[1]:
                continue
            a2 = max(0, e[1] + a1 + f[2] - 128)
            if a2 > f[2]:
                continue
            a3 = max(0, e[2] + a2 + f[3] - 128)
            if a3 > f[3]:
                continue
            if e[3] + a3 + f[0] - a0 <= 128:
                sol = [a0, a1, a2, a3]
                break
        if sol is None:
            take -= 1
            continue
        # assign windows
        win = np.array(k, np.int64)  # exclusive default: window k
        for kk in range(NW):
            idxs = np.nonzero((k == kk) & fl)[0]
            nup = sol[kk]
            win[idxs[:nup]] = kk                 # stay in window kk
            win[idxs[nup:]] = (kk - 1) % NW      # spill down to kk-1
        return take, win
    return 0, np.zeros(0, np.int64)


def _prep_core(ii, jj, cfg):
    """Slot assignment for one core. ii/jj: this core's edges (sorted by ii).
    Returns slot_edge [CAP] (edge idx into ii/jj or -1), widx [CAP] int16,
    bases [NH]."""
    d = _derived(cfg)
    HALF, NH, NW, CAP = cfg["HALF"], cfg["NH"], cfg["NW"], d["CAP"]
    WSTRIDE, N = cfg["WSTRIDE"], cfg["N_ATOMS"]
    ne = len(ii)
    slot_edge = np.full(CAP, -1, np.int64)
    widx = np.zeros(CAP, np.int16)
    bases = np.zeros(NH, np.int64)
    ptr = 0
    last_base = 0
    for h in range(NH):
        take = min(HALF, ne - ptr)
        if take > 0:
            n, win = _solve_half(ii[ptr : ptr + take], jj[ptr : ptr + take], cfg, take)
        else:
            n, win = 0, np.zeros(0, np.int64)
        base = int(ii[ptr]) if n > 0 else last_base
        bases[h] = base
        last_base = base
        s0 = h * HALF
        for kk in range(NW):
            sel = np.nonzero(win == kk)[0]
            lw = ((jj[ptr + sel] - WSTRIDE * kk) % N).astype(np.int16)
            order = np.argsort(lw, kind="stable")
            sel, lw = sel[order], lw[order]
            blk = s0 + kk * 128
            slot_edge[blk : blk + len(sel)] = ptr + sel
            widx[blk : blk + len(sel)] = lw
            widx[blk + len(sel) : blk + 128] = 0
        ptr += n
    if ptr != ne:
        raise RuntimeError(f"slot assignment overflow: {ne - ptr} edges left")
    return slot_edge, widx, bases


def _chunk_position_perm(cfg):
    """Permutation mapping 'half-major' slot index -> 'device position'.
    Device position order: per chunk, window-major then half then 128-block.
    Returns pos[s_halfmajor] = device position."""
    d = _derived(cfg)
    HALF, NW = cfg["HALF"], cfg["NW"]
    pos = np.empty(d["CAP"], np.int64)
    B = 0
    h0 = 0
    for nh in cfg["CHUNK_HALVES"]:
        for hl in range(nh):
            for k in range(NW):
                src = (h0 + hl) * HALF + k * 128
                dst = B + k * (nh * 128) + hl * 128
                pos[src : src + 128] = np.arange(dst, dst + 128)
        B += nh * HALF
        h0 += nh
    return pos


def _host_fallback(x, Wij, idx_i, idx_j, N, F):
    ii = np.asarray(idx_i, np.int64)
    jj = np.asarray(idx_j, np.int64)
    prod = x[jj] * Wij
    if len(ii) and np.all(ii[:-1] <= ii[1:]):
        starts = np.searchsorted(ii, np.arange(N), side="left")
        ends = np.append(starts[1:], len(ii))
        y = np.add.reduceat(prod, np.minimum(starts, len(ii) - 1), axis=0)
        y[starts >= ends] = 0
        return y.astype(np.float32)
    y = np.zeros((N, F), np.float32)
    np.add.at(y, ii, prod)
    return y


def kernel(x, Wij, idx_i, idx_j):
    global last_results
    from concourse import bass_utils

    cfg = CFG
    d = _derived(cfg)
    N, F, E, NC = cfg["N_ATOMS"], cfg["F"], cfg["E"], cfg["NCORES"]
    CAP, NCOLS, NH, HALF = d["CAP"], d["NCOLS"], cfg["NH"], cfg["HALF"]
    NW, WSTRIDE, WREACH, OHW = cfg["NW"], cfg["WSTRIDE"], cfg["WREACH"], cfg["OHW"]
    XPAD, FLEX = cfg["XPAD"], d["FLEX"]

    x = np.ascontiguousarray(np.asarray(x), dtype=np.float32)
    Wij = np.ascontiguousarray(np.asarray(Wij), dtype=np.float32)
    ii = np.asarray(idx_i, dtype=np.int64)
    jj = np.asarray(idx_j, dtype=np.int64)
    ok = (
        x.shape == (N, F)
        and Wij.shape == (E, F)
        and ii.shape == (E,)
        and np.all(ii[:-1] <= ii[1:])
        and ii.min() >= 0
        and ii.max() < N
        and jj.min() >= 0
        and jj.max() < N
    )
    if not ok:
        return _host_fallback(x, Wij, ii, jj, N, F)

    if "nc" not in _CACHE:
        _CACHE["nc"] = _build_program(cfg)
        _CACHE["pos"] = _chunk_position_perm(cfg)
        _CACHE["colh"] = _half_of_position(cfg)
    nc = _CACHE["nc"]
    pos = _CACHE["pos"]
    colh = _CACHE["colh"]

    # x windows: bf16 rows padded to 256B stride (with wraparound replica)
    x_bf = x.astype(BF16)
    x_pad = np.zeros((N + FLEX, XPAD), BF16)
    x_pad[:N, :F] = x_bf
    x_pad[N:, :F] = x_bf[:FLEX]
    wins = [
        np.ascontiguousarray(x_pad[k * WSTRIDE : k * WSTRIDE + WREACH])
        for k in range(NW)
    ]
    iota_arr = np.ascontiguousarray(
        np.broadcast_to(np.arange(OHW, dtype=np.float32), (128, OHW))
    ).astype(BF16)
    Wij_bf = Wij.astype(BF16)
    Wij_pad = np.concatenate([Wij_bf, np.zeros((1, F), BF16)], axis=0)

    EC = E // NC
    in_maps = []
    all_bases = []
    try:
        for c in range(NC):
            iic = ii[c * EC : (c + 1) * EC]
            jjc = jj[c * EC : (c + 1) * EC]
            slot_edge_h, widx_h, bases = _prep_core(iic, jjc, cfg)
            # to device position order
            slot_edge = np.empty(CAP, np.int64)
            widx = np.empty(CAP, np.int16)
            slot_edge[pos] = slot_edge_h
            widx[pos] = widx_h
            # rr in device order
            ge = np.where(slot_edge >= 0, slot_edge, 0)
            rr_flat = iic[ge].astype(np.float32)
            rr_flat -= bases[colh]
            rr_flat[slot_edge < 0] = -1.0
            span_ok = (rr_flat[slot_edge >= 0] >= 0).all() and (
                rr_flat[slot_edge >= 0] < OHW
            ).all()
            if not span_ok:
                raise RuntimeError("rr out of range")
            # Wij into [128, NCOLS*F] (slot (p,c) = device position c*128+p)
            gedge = np.where(slot_edge >= 0, c * EC + slot_edge, E)
            wsl = Wij_pad[gedge]  # [CAP, F]
            wij_arr = np.ascontiguousarray(
                wsl.reshape(NCOLS, 128, F).transpose(1, 0, 2).reshape(128, NCOLS * F)
            )
            rr_arr = np.ascontiguousarray(rr_flat.reshape(NCOLS, 128).T).astype(BF16)
            idx16 = _arrange_idx16(widx, cfg)
            m = {"wij": wij_arr, "rr": rr_arr, "idx16": idx16, "iota": iota_arr}
            for k in range(NW):
                m[f"w{k}"] = wins[k]
            in_maps.append(m)
            all_bases.append(bases)
    except RuntimeError:
        return _host_fallback(x, Wij, ii, jj, N, F)

    res = None
    for attempt in range(3):
        try:
            res = bass_utils.run_bass_kernel_spmd(
                nc, in_maps, core_ids=list(range(NC))
            )
            break
        except Exception:
            import time as _time

            _time.sleep(5 * (attempt + 1))
    if res is None:
        return _host_fallback(x, Wij, ii, jj, N, F)
    last_results = res

    y = np.zeros((N + OHW, F), np.float32)
    for c in range(NC):
        P = res.results[c]["out"].reshape(128, NH // 2, F)
        b = all_bases[c]
        for g in range(NH // 2):
            y[b[2 * g] : b[2 * g] + OHW] += P[0:OHW, g, :]
            y[b[2 * g + 1] : b[2 * g + 1] + OHW] += P[OHW:128, g, :]
    return y[:N]


def _half_of_position(cfg):
    """half id for each device position."""
    d = _derived(cfg)
    out = np.empty(d["CAP"], np.int64)
    B = 0
    h0 = 0
    for nh in cfg["CHUNK_HALVES"]:
        for k in range(cfg["NW"]):
            for hl in range(nh):
                dst = B + k * (nh * 128) + hl * 128
                out[dst : dst + 128] = h0 + hl
        B += nh * cfg["HALF"]
        h0 += nh
    return out


def _arrange_idx16(widx, cfg):
    """widx in device position order [CAP] -> [128, IDXCOLS] int16 wrapped
    (idx r at [r%16, r//16] within each call, replicated x8 down partitions)."""
    d = _derived(cfg)
    cols = []
    B = 0
    for nh in cfg["CHUNK_HALVES"]:
        num = nh * 128
        for k in range(cfg["NW"]):
            vals = widx[B + k * num : B + (k + 1) * num]
            w = vals.reshape(num // 16, 16).T  # [16, num/16]
            cols.append(np.tile(w, (8, 1)))
        B += cfg["NW"] * num
    return np.ascontiguousarray(np.concatenate(cols, axis=1))


# revision 13
# speedup vs baseline: 1.0267x; 1.0267x over previous
"""CFConv (GNN message passing) on 8 Trainium2 cores.

    y = segment_sum(x[idx_j] * Wij, idx_i)   with idx_i sorted

Device strategy (uniform SPMD program, per-core data):
  - Edges sharded contiguously across 8 cores (idx_i sorted => contiguous
    atom ranges; boundary overlaps fixed host-side).
  - Per core, edges are packed into 512-edge "halves" (atom span <= 64,
    verified; pad slots inserted where needed). Each half's 512 slots are
    split into exactly 128 slots per x-"window" (4 overlapping windows of
    32768 rows, stride 25000, wraparound replica) so the gather can use the
    custom int16 dma_gather instruction (4 SWDGE queues).
  - x windows stored in DRAM as bf16 rows PADDED to 256B (row stride 128
    bf16, 64 useful). The gather uses elem_size=64 bf16 (128B descriptors)
    with elem_step=128 (256B stride, the ucode's granularity): half the DMA
    bus time of f32 rows. Gather calls are one per (chunk, window) with
    nh*128 indices (up to 4096) to amortize the ~1us fixed GPSIMD
    descriptor-generation overhead per call.
  - x_j * Wij elementwise on VectorE in bf16 (Wij host-relaid into slot
    order, bf16).
  - Segment-sum via one-hot matmul in bf16: rr = idx_i - half_base in
    [0,64); VectorE builds one-hot (rr == iota) in bf16; TensorE
    accumulates K=128 bf16 matmuls into per-half [64,64] f32 PSUM frames
    (2 frames per 128-row psum block).
  - Device emits dense per-group f32 partials; host adds each [64,F] frame
    into y at its half's base atom (~2 overlapping partials per atom).
"""

import sys

import numpy as np

if "/opt/trn_rl_repo" not in sys.path:
    sys.path.insert(0, "/opt/trn_rl_repo")

import ml_dtypes

BF16 = ml_dtypes.bfloat16

CFG = dict(
    N_ATOMS=100000,
    F=64,
    E=1250000,
    NCORES=8,
    HALF=512,          # edges per half-group
    OHW=64,            # one-hot width (max atom span per half; data max ~50)
    NW=4,              # x windows
    WSTRIDE=25000,     # window stride (NW*WSTRIDE == N_ATOMS)
    WREACH=32768,      # window size (int16 index limit)
    XPAD=128,          # bf16 row stride in padded x windows (256B)
    NH=308,            # halves per core
    # sums to NH; nh*128 idxs per gather call. Mid chunks large to amortize
    # the ~0.5us fixed GPSIMD cost per gather call; last chunks small so the
    # post-gather compute tail after the final gather is short.
    CHUNK_HALVES=[32] * 9 + [16, 4],
)

_CACHE = {}
last_results = None


def _derived(cfg):
    d = dict(cfg)
    d["CAP"] = cfg["NH"] * cfg["HALF"]
    d["NCOLS"] = d["CAP"] // 128
    d["NGROUPS"] = cfg["NH"] // 2
    d["FLEX"] = cfg["WREACH"] - cfg["WSTRIDE"]
    d["IDXCOLS"] = sum(4 * (nh * 128) // 16 for nh in cfg["CHUNK_HALVES"])
    assert sum(cfg["CHUNK_HALVES"]) == cfg["NH"]
    assert cfg["NW"] * cfg["WSTRIDE"] == cfg["N_ATOMS"]
    return d


def _dma_gather_strided(eng, out_ap, in_ap, idxs_ap, num_idxs, elem_size,
                        elem_step, queue_num):
    """nc.gpsimd.dma_gather (non-transpose, DRAM source) without the
    elem_size_bytes % 256 restriction. The ucode's non-transpose path only
    needs the row STRIDE 256B-granular (stride_bytes_256 encoding); the
    payload per descriptor can be any size (here 128B = one bf16 row)."""
    import concourse.mybir as mybir
    from concourse._compat import exact_div

    eng._assert_queue_num(queue_num)
    assert idxs_ap.dtype == mybir.dt.int16
    assert in_ap.dtype == out_ap.dtype
    assert in_ap.ap[0][0] == elem_step
    assert in_ap.ap[-1][1] == out_ap.ap[-1][1] == elem_size
    assert out_ap.ap[0][1] * out_ap.ap[1][1] == ((num_idxs + 127) // 128) * 128
    stride_bytes = elem_step * mybir.dt.size(in_ap.dtype)
    stride_bytes_256 = exact_div(stride_bytes, 256)
    assert stride_bytes_256 < 256
    _in_ap = eng.lower_ap_dma(in_ap, for_custom_bir_dma=True)
    _idxs_ap = eng.lower_ap(idxs_ap)
    _out_ap = eng.lower_ap(out_ap)
    return eng.add_instruction(
        mybir.InstDMAGatherAnt(
            name=eng.bass.get_next_instruction_name(),
            ins=[
                *_in_ap,
                _idxs_ap,
                eng.lower_val_access(eng.to_reg(num_idxs)),
            ],
            outs=[_out_ap],
            transpose=False,
            num_idxs=num_idxs,
            elem_size=elem_size,
            stride_bytes_256=stride_bytes_256,
            gen_mode=0,
            single_packet=False,
            queue_num=queue_num,
        )
    )


def _build_program(cfg):
    import concourse.bacc as bacc
    import concourse.tile as tile
    import concourse.mybir as mybir
    from concourse.library_config import mlp

    d = _derived(cfg)
    F, NW, XPAD = cfg["F"], cfg["NW"], cfg["XPAD"]
    NCOLS, IDXCOLS, NGROUPS = d["NCOLS"], d["IDXCOLS"], d["NGROUPS"]
    WREACH, OHW = cfg["WREACH"], cfg["OHW"]

    nc = bacc.Bacc(
        "TRN2",
        target_bir_lowering=False,
        num_swdge_queues=4,
        dynamic_dma_scratch_size=32768,
    )
    f32 = mybir.dt.float32
    bf16 = mybir.dt.bfloat16
    wij_d = nc.dram_tensor("wij", [128, NCOLS * F], bf16, kind="ExternalInput")
    win_d = [
        nc.dram_tensor(f"w{k}", [WREACH, XPAD], bf16, kind="ExternalInput")
        for k in range(NW)
    ]
    idx_d = nc.dram_tensor("idx16", [128, IDXCOLS], mybir.dt.int16, kind="ExternalInput")
    rr_d = nc.dram_tensor("rr", [128, NCOLS], bf16, kind="ExternalInput")
    iota_d = nc.dram_tensor("iota", [128, OHW], bf16, kind="ExternalInput")
    out_d = nc.dram_tensor("out", [128, NGROUPS * F], f32, kind="ExternalOutput")

    with tile.TileContext(nc) as tc:
        with (
            tc.tile_pool(name="const", bufs=1) as cpool,
            tc.tile_pool(name="idx", bufs=3) as ipool,
            tc.tile_pool(name="wij", bufs=3) as wpool,
            tc.tile_pool(name="gx", bufs=3) as gpool,
            tc.tile_pool(name="oh", bufs=3) as spool,
            tc.tile_pool(name="stage", bufs=3) as opool,
            tc.tile_pool(name="psum", bufs=3, space="PSUM") as ppool,
        ):
            iota_t = cpool.tile([128, OHW], bf16)
            nc.sync.dma_start(out=iota_t[:], in_=iota_d[:])
            rr_t = cpool.tile([128, NCOLS], bf16)
            nc.sync.dma_start(out=rr_t[:], in_=rr_d[:])

            # chunk-0 input DMAs issued before the critical section so the
            # transfers overlap the ~17us GPSIMD library load
            nh0 = cfg["CHUNK_HALVES"][0]
            idx_t0 = ipool.tile([128, 4 * (nh0 * 8)], mybir.dt.int16, tag="idx")
            nc.sync.dma_start(out=idx_t0[:], in_=idx_d[:, : 4 * (nh0 * 8)])
            wij_sb0 = wpool.tile([128, 4 * nh0 * F], bf16, tag="wij")
            nc.sync.dma_start(out=wij_sb0[:], in_=wij_d[:, : 4 * nh0 * F])

            with tc.tile_critical():
                nc.gpsimd.load_library(mlp)

            iota_b = iota_t[:].rearrange("p (o f) -> p o f", o=1)

            col0 = 0   # global column base of chunk
            ix0 = 0    # global idx16 column base
            for ci, nh in enumerate(cfg["CHUNK_HALVES"]):
                ncols = 4 * nh              # columns in this chunk
                num = nh * 128              # indices per gather call
                icols = num // 16           # idx16 cols per window call
                if ci == 0:
                    idx_t, wij_sb = idx_t0, wij_sb0
                else:
                    idx_t = ipool.tile([128, 4 * icols], mybir.dt.int16, tag="idx")
                    nc.sync.dma_start(
                        out=idx_t[:], in_=idx_d[:, ix0 : ix0 + 4 * icols]
                    )
                    wij_sb = wpool.tile([128, ncols * F], bf16, tag="wij")
                    nc.sync.dma_start(
                        out=wij_sb[:],
                        in_=wij_d[:, col0 * F : (col0 + ncols) * F],
                    )
                gx = gpool.tile([128, ncols * F], bf16, tag="gx")
                for k in range(NW):
                    _dma_gather_strided(
                        nc.gpsimd,
                        out_ap=gx[:, (k * nh) * F : (k + 1) * nh * F].rearrange(
                            "p (c f) -> p c f", f=F
                        ),
                        in_ap=win_d[k][:, 0:F],
                        idxs_ap=idx_t[:, k * icols : (k + 1) * icols],
                        num_idxs=num,
                        elem_size=F,
                        elem_step=XPAD,
                        queue_num=k,
                    )
                # one-hot first: it does not depend on the gather, so the
                # Vector engine can run ahead of the gather stream
                s_t = spool.tile([128, ncols * OHW], bf16, tag="oh")
                nc.vector.tensor_tensor(
                    out=s_t[:],
                    in0=iota_b.to_broadcast([128, ncols, OHW]),
                    in1=rr_t[:, col0 : col0 + ncols].to_broadcast([128, ncols, OHW]),
                    op=mybir.AluOpType.is_equal,
                )
                nc.vector.tensor_tensor(
                    out=gx[:],
                    in0=gx[:],
                    in1=wij_sb[:],
                    op=mybir.AluOpType.mult,
                )
                pt = ppool.tile([128, (nh // 2) * F], f32, tag="ps")
                for hl in range(nh):
                    for k in range(NW):
                        c = k * nh + hl
                        nc.tensor.matmul(
                            out=pt[
                                (hl % 2) * OHW : (hl % 2 + 1) * OHW,
                                (hl // 2) * F : (hl // 2 + 1) * F,
                            ],
                            lhsT=s_t[:, c * OHW : (c + 1) * OHW],
                            rhs=gx[:, c * F : (c + 1) * F],
                            start=(k == 0),
                            stop=(k == NW - 1),
                        )
                stage = opool.tile([128, (nh // 2) * F], f32, tag="st")
                nc.scalar.copy(out=stage[:], in_=pt[:])
                g0 = col0 // 8  # global group base (col0 = sum 4*nh, groups nh/2)
                nc.sync.dma_start(
                    out=out_d[:, g0 * F : (g0 + nh // 2) * F],
                    in_=stage[:],
                )
                col0 += ncols
                ix0 += 4 * icols

    nc.compile()
    return nc


def _solve_half(ai, aj, cfg, take):
    """Pick the largest prefix (<= take) of this half's candidate edges that
    satisfies span<OHW and the per-window capacity-128 balance; returns
    (n_taken, win_assign[n]) or reduces take."""
    WSTRIDE, FLEX, NW, OHW = cfg["WSTRIDE"], cfg["WREACH"] - cfg["WSTRIDE"], cfg["NW"], cfg["OHW"]
    while take > 0:
        a = ai[:take]
        if a[-1] - a[0] >= OHW:
            # cut to span
            take = int(np.searchsorted(a, a[0] + OHW, side="left"))
            continue
        j = aj[:take]
        k = j // WSTRIDE
        fl = (j % WSTRIDE) < FLEX
        e = np.zeros(NW, np.int64)
        f = np.zeros(NW, np.int64)
        for kk in range(NW):
            e[kk] = int(((k == kk) & ~fl).sum())
            f[kk] = int(((k == kk) & fl).sum())
        if e.max() > 128:
            take -= 1
            continue
        sol = None
        for a0 in range(int(f[0]) + 1):
            a1 = max(0, e[0] + a0 + f[1] - 128)
            if a1 > f[1]:
                continue
            a2 = max(0, e[1] + a1 + f[2] - 128)
            if a2 > f[2]:
                continue
            a3 = max(0, e[2] + a2 + f[3] - 128)
            if a3 > f[3]:
                continue
            if e[3] + a3 + f[0] - a0 <= 128:
                sol = [a0, a1, a2, a3]
                break
        if sol is None:
            take -= 1
            continue
        # assign windows
        win = np.array(k, np.int64)  # exclusive default: window k
        for kk in range(NW):
            idxs = np.nonzero((k == kk) & fl)[0]
            nup = sol[kk]
            win[idxs[:nup]] = kk                 # stay in window kk
            win[idxs[nup:]] = (kk - 1) % NW      # spill down to kk-1
        return take, win
    return 0, np.zeros(0, np.int64)


def _prep_core(ii, jj, cfg):
    """Slot assignment for one core. ii/jj: this core's edges (sorted by ii).
    Returns slot_edge [CAP] (edge idx into ii/jj or -1), widx [CAP] int16,
    bases [NH]."""
    d = _derived(cfg)
    HALF, NH, NW, CAP = cfg["HALF"], cfg["NH"], cfg["NW"], d["CAP"]
    WSTRIDE, N = cfg["WSTRIDE"], cfg["N_ATOMS"]
    ne = len(ii)
    slot_edge = np.full(CAP, -1, np.int64)
    widx = np.zeros(CAP, np.int16)
    bases = np.zeros(NH, np.int64)
    ptr = 0
    last_base = 0
    for h in range(NH):
        take = min(HALF, ne - ptr)
        if take > 0:
            n, win = _solve_half(ii[ptr : ptr + take], jj[ptr : ptr + take], cfg, take)
        else:
            n, win = 0, np.zeros(0, np.int64)
        base = int(ii[ptr]) if n > 0 else last_base
        bases[h] = base
        last_base = base
        s0 = h * HALF
        for kk in range(NW):
            sel = np.nonzero(win == kk)[0]
            lw = ((jj[ptr + sel] - WSTRIDE * kk) % N).astype(np.int16)
            order = np.argsort(lw, kind="stable")
            sel, lw = sel[order], lw[order]
            blk = s0 + kk * 128
            slot_edge[blk : blk + len(sel)] = ptr + sel
            widx[blk : blk + len(sel)] = lw
            widx[blk + len(sel) : blk + 128] = 0
        ptr += n
    if ptr != ne:
        raise RuntimeError(f"slot assignment overflow: {ne - ptr} edges left")
    return slot_edge, widx, bases


def _chunk_position_perm(cfg):
    """Permutation mapping 'half-major' slot index -> 'device position'.
    Device position order: per chunk, window-major then half then 128-block.
    Returns pos[s_halfmajor] = device position."""
    d = _derived(cfg)
    HALF, NW = cfg["HALF"], cfg["NW"]
    pos = np.empty(d["CAP"], np.int64)
    B = 0
    h0 = 0
    for nh in cfg["CHUNK_HALVES"]:
        for hl in range(nh):
            for k in range(NW):
                src = (h0 + hl) * HALF + k * 128
                dst = B + k * (nh * 128) + hl * 128
                pos[src : src + 128] = np.arange(dst, dst + 128)
        B += nh * HALF
        h0 += nh
    return pos


def _host_fallback(x, Wij, idx_i, idx_j, N, F):
    ii = np.asarray(idx_i, np.int64)
    jj = np.asarray(idx_j, np.int64)
    prod = x[jj] * Wij
    if len(ii) and np.all(ii[:-1] <= ii[1:]):
        starts = np.searchsorted(ii, np.arange(N), side="left")
        ends = np.append(starts[1:], len(ii))
        y = np.add.reduceat(prod, np.minimum(starts, len(ii) - 1), axis=0)
        y[starts >= ends] = 0
        return y.astype(np.float32)
    y = np.zeros((N, F), np.float32)
    np.add.at(y, ii, prod)
    return y


def kernel(x, Wij, idx_i, idx_j):
    global last_results
    from concourse import bass_utils

    cfg = CFG
    d = _derived(cfg)
    N, F, E, NC = cfg["N_ATOMS"], cfg["F"], cfg["E"], cfg["NCORES"]
    CAP, NCOLS, NH, HALF = d["CAP"], d["NCOLS"], cfg["NH"], cfg["HALF"]
    NW, WSTRIDE, WREACH, OHW = cfg["NW"], cfg["WSTRIDE"], cfg["WREACH"], cfg["OHW"]
    XPAD, FLEX = cfg["XPAD"], d["FLEX"]

    x = np.ascontiguousarray(np.asarray(x), dtype=np.float32)
    Wij = np.ascontiguousarray(np.asarray(Wij), dtype=np.float32)
    ii = np.asarray(idx_i, dtype=np.int64)
    jj = np.asarray(idx_j, dtype=np.int64)
    ok = (
        x.shape == (N, F)
        and Wij.shape == (E, F)
        and ii.shape == (E,)
        and np.all(ii[:-1] <= ii[1:])
        and ii.min() >= 0
        and ii.max() < N
        and jj.min() >= 0
        and jj.max() < N
    )
    if not ok:
        return _host_fallback(x, Wij, ii, jj, N, F)

    if "nc" not in _CACHE:
        _CACHE["nc"] = _build_program(cfg)
        _CACHE["pos"] = _chunk_position_perm(cfg)
        _CACHE["colh"] = _half_of_position(cfg)
    nc = _CACHE["nc"]
    pos = _CACHE["pos"]
    colh = _CACHE["colh"]

    # x windows: bf16 rows padded to 256B stride (with wraparound replica)
    x_bf = x.astype(BF16)
    x_pad = np.zeros((N + FLEX, XPAD), BF16)
    x_pad[:N, :F] = x_bf
    x_pad[N:, :F] = x_bf[:FLEX]
    wins = [
        np.ascontiguousarray(x_pad[k * WSTRIDE : k * WSTRIDE + WREACH])
        for k in range(NW)
    ]
    iota_arr = np.ascontiguousarray(
        np.broadcast_to(np.arange(OHW, dtype=np.float32), (128, OHW))
    ).astype(BF16)
    Wij_bf = Wij.astype(BF16)
    Wij_pad = np.concatenate([Wij_bf, np.zeros((1, F), BF16)], axis=0)

    EC = E // NC
    in_maps = []
    all_bases = []
    try:
        for c in range(NC):
            iic = ii[c * EC : (c + 1) * EC]
            jjc = jj[c * EC : (c + 1) * EC]
            slot_edge_h, widx_h, bases = _prep_core(iic, jjc, cfg)
            # to device position order
            slot_edge = np.empty(CAP, np.int64)
            widx = np.empty(CAP, np.int16)
            slot_edge[pos] = slot_edge_h
            widx[pos] = widx_h
            # rr in device order
            ge = np.where(slot_edge >= 0, slot_edge, 0)
            rr_flat = iic[ge].astype(np.float32)
            rr_flat -= bases[colh]
            rr_flat[slot_edge < 0] = -1.0
            span_ok = (rr_flat[slot_edge >= 0] >= 0).all() and (
                rr_flat[slot_edge >= 0] < OHW
            ).all()
            if not span_ok:
                raise RuntimeError("rr out of range")
            # Wij into [128, NCOLS*F] (slot (p,c) = device position c*128+p)
            gedge = np.where(slot_edge >= 0, c * EC + slot_edge, E)
            wsl = Wij_pad[gedge]  # [CAP, F]
            wij_arr = np.ascontiguousarray(
                wsl.reshape(NCOLS, 128, F).transpose(1, 0, 2).reshape(128, NCOLS * F)
            )
            rr_arr = np.ascontiguousarray(rr_flat.reshape(NCOLS, 128).T).astype(BF16)
            idx16 = _arrange_idx16(widx, cfg)
            m = {"wij": wij_arr, "rr": rr_arr, "idx16": idx16, "iota": iota_arr}
            for k in range(NW):
                m[f"w{k}"] = wins[k]
            in_maps.append(m)
            all_bases.append(bases)
    except RuntimeError:
        return _host_fallback(x, Wij, ii, jj, N, F)

    res = None
    for attempt in range(3):
        try:
            res = bass_utils.run_bass_kernel_spmd(
                nc, in_maps, core_ids=list(range(NC))
            )
            break
        except Exception:
            import time as _time

            _time.sleep(5 * (attempt + 1))
    if res is None:
        return _host_fallback(x, Wij, ii, jj, N, F)
    last_results = res

    y = np.zeros((N + OHW, F), np.float32)
    for c in range(NC):
        P = res.results[c]["out"].reshape(128, NH // 2, F)
        b = all_bases[c]
        for g in range(NH // 2):
            y[b[2 * g] : b[2 * g] + OHW] += P[0:OHW, g, :]
            y[b[2 * g + 1] : b[2 * g + 1] + OHW] += P[OHW:128, g, :]
    return y[:N]


def _half_of_position(cfg):
    """half id for each device position."""
    d = _derived(cfg)
    out = np.empty(d["CAP"], np.int64)
    B = 0
    h0 = 0
    for nh in cfg["CHUNK_HALVES"]:
        for k in range(cfg["NW"]):
            for hl in range(nh):
                dst = B + k * (nh * 128) + hl * 128
                out[dst : dst + 128] = h0 + hl
        B += nh * cfg["HALF"]
        h0 += nh
    return out


def _arrange_idx16(widx, cfg):
    """widx in device position order [CAP] -> [128, IDXCOLS] int16 wrapped
    (idx r at [r%16, r//16] within each call, replicated x8 down partitions)."""
    d = _derived(cfg)
    cols = []
    B = 0
    for nh in cfg["CHUNK_HALVES"]:
        num = nh * 128
        for k in range(cfg["NW"]):
            vals = widx[B + k * num : B + (k + 1) * num]
            w = vals.reshape(num // 16, 16).T  # [16, num/16]
            cols.append(np.tile(w, (8, 1)))
        B += cfg["NW"] * num
    return np.ascontiguousarray(np.concatenate(cols, axis=1))
